# revision 41
# baseline (speedup 1.0000x reference)
"""TRN2 Bass kernel for nn_IrrepsLinear (e3nn-style per-irrep linear layer).

Computation (per node n, N=200000 nodes, 480 features):
  out0 = (x0 @ W0 + modal_attr[batch[n]] @ W0m) / sqrt(130)   cols   0:128
  out1 = einsum('nim,io->nom', x1, W1) / sqrt(64)             cols 128:320
  out2 = einsum('nim,io->nom', x2, W2) / sqrt(32)             cols 320:480

Strategy: data-parallel over nodes across 8 NeuronCores (25000 nodes/core,
padded to 25088 = 49 macros of 512 nodes = 196 blocks of 128 nodes).

Host-side layout tricks that minimize PE work (matmul cost on TRN2 is
~1 cycle per OUTPUT column at fp16, independent of K):
  - features are permuted to m-major order (0e | 1e m-major | 2e m-major),
    which makes the 480x480 block-diagonal weight align exactly with 128-
    feature chunks: Wc0=W0, Wc1=diag(W1,W1), Wc2=diag(W2 x4), Wc3=diag(W2 x3).
    Each 128-col output chunk then needs exactly ONE matmul (no 2-pass
    accumulation as with the interleaved layout).
  - x is pre-transposed on host (chunk-major [feature, node] layout) so the
    PE does zero transposes; x chunks feed matmuls directly as the stationary
    operand.
  - batch is sorted, so each core's 25000-node shard spans < 128 group ids;
    host subtracts a per-core window base w0 and ships the 128-row window of
    modal_attr, so the modal gather is ONE one-hot matmul per block:
      out0 += onehot128(batch - w0) @ G,  G = mattr[w0:w0+128] @ (W0m/sqrt130)
    (G computed on device via a K=2 matmul).

Per 128-node block the PE does 5 matmuls (4 chunks + modal one-hot), 608
output columns total. The one-hot is built per 512-node macro by a GPSIMD
partition_broadcast of the batch ids + one DVE fp16 is_equal against a
per-partition iota. PSUM discipline per block: chunk0 start=True (marks the
2KB zero region), chunks 1-3 and modal start=False (first write to a
pending-zero byte overwrites, modal accumulates onto chunk0's columns).

fp16 throughout the datapath (fp32 PSUM accumulate): x loads on the SWDGE
queue, outputs on HWDGE; psum->SBUF fp16 casts alternate DVE/ACT.
"""
import numpy as np

import concourse.bass as bass
import concourse.mybir as mybir
import concourse.tile as tile
from concourse import bacc
from concourse.bass_utils import run_bass_kernel_spmd

f32 = mybir.dt.float32
f16 = mybir.dt.float16
i32 = mybir.dt.int32

N_CORES = 8
MUL0, MUL1, MUL2, NMOD = 128, 64, 32, 2
DTOT = 480
NODES_PER_MACRO = 512
NB = 4                      # 128-node blocks per macro
MACROS_PER_SUPER = 7        # 7.2KB/partition input descriptors
KC = (128, 128, 128, 96)    # contraction size per feature chunk


def _perm():
    """new -> old feature index: [0e | 1e m-major | 2e m-major]."""
    p = np.empty(DTOT, dtype=np.int64)
    p[0:128] = np.arange(128)
    for m in range(3):
        p[128 + m * MUL1:128 + (m + 1) * MUL1] = 128 + np.arange(MUL1) * 3 + m
    for m in range(5):
        p[320 + m * MUL2:320 + (m + 1) * MUL2] = 320 + np.arange(MUL2) * 5 + m
    return p


def _build_wchunks(W0, W1, W2):
    inv0 = np.float32(1.0) / np.sqrt(np.float32(MUL0 + NMOD))
    inv1 = np.float32(1.0) / np.sqrt(np.float32(MUL1))
    inv2 = np.float32(1.0) / np.sqrt(np.float32(MUL2))
    wc0 = np.ascontiguousarray(W0 * inv0)
    wc1 = np.zeros((128, 128), dtype=np.float32)
    wc1[0:64, 0:64] = W1 * inv1
    wc1[64:128, 64:128] = W1 * inv1
    wc2 = np.zeros((128, 128), dtype=np.float32)
    wc2[0:64, 0:64] = W1 * inv1
    for m in range(2):
        wc2[64 + 32 * m:64 + 32 * (m + 1),
            64 + 32 * m:64 + 32 * (m + 1)] = W2 * inv2
    wc3 = np.zeros((96, 96), dtype=np.float32)
    for m in range(3):
        wc3[32 * m:32 * (m + 1), 32 * m:32 * (m + 1)] = W2 * inv2
    return wc0, wc1, wc2, wc3


def _host_prep(x, modal_attr, W0, W0m, W1, W2, batch):
    x = np.asarray(x)
    N = x.shape[0]
    ns = N // N_CORES
    ns_pad = ((ns + NODES_PER_MACRO - 1) // NODES_PER_MACRO) * NODES_PER_MACRO
    perm = _perm()
    wc0, wc1, wc2, wc3 = _build_wchunks(
        np.asarray(W0, dtype=np.float32), np.asarray(W1, dtype=np.float32),
        np.asarray(W2, dtype=np.float32))
    inv0 = np.float32(1.0) / np.sqrt(np.float32(MUL0 + NMOD))
    w0m_s = np.ascontiguousarray(np.asarray(W0m, dtype=np.float32) * inv0)
    mattrT = np.asarray(modal_attr, dtype=np.float32).T  # [NMOD, 512]
    batch = np.asarray(batch)
    ngrp = mattrT.shape[1]

    in_maps = []
    for i in range(N_CORES):
        bsh = batch[i * ns:(i + 1) * ns]
        g0 = int(bsh[0])
        w0 = min(g0, max(ngrp - 128, 0))
        assert int(bsh[-1]) - w0 < 128, (
            f"core {i}: batch window [{g0},{int(bsh[-1])}] spans >=128 groups")
        bs = np.zeros((ns_pad,), dtype=np.float16)
        bs[:ns] = (bsh - w0).astype(np.float16)

        xpad = np.zeros((ns_pad, DTOT), dtype=np.float16)
        xpad[:ns] = x[i * ns:(i + 1) * ns][:, perm]
        xt = np.zeros((128, 4 * ns_pad), dtype=np.float16)
        for c in range(4):
            xt[0:KC[c], c * ns_pad:(c + 1) * ns_pad] = \
                xpad[:, 128 * c:128 * c + KC[c]].T

        # all small constants in one fp16 blob -> single preamble DMA
        wblob = np.zeros((128, 736), dtype=np.float16)
        wblob[:, 0:128] = wc0
        wblob[:, 128:256] = wc1
        wblob[:, 256:384] = wc2
        wblob[0:96, 384:480] = wc3
        wblob[0:NMOD, 480:608] = w0m_s
        wblob[0:NMOD, 608:736] = mattrT[:, w0:w0 + 128]

        in_maps.append({"xt": xt, "bs": bs, "wblob": wblob})
    return in_maps, ns, ns_pad


def _build_nc(ns_pad):
    nmacro = ns_pad // NODES_PER_MACRO
    nodes_sup = MACROS_PER_SUPER * NODES_PER_MACRO          # 3584
    nblk = ns_pad // 128
    # taper the tail so the compute backlog at input-stream end is only a
    # few macros and the last outputs drain nearly flush with compute
    supers = []
    m0 = 0
    while nmacro - m0 > 14:
        supers.append((m0, MACROS_PER_SUPER))
        m0 += MACROS_PER_SUPER
    rem = nmacro - m0
    tail_sizes = {14: (5, 4, 2, 1, 1, 1), 7: (3, 2, 1, 1)}.get(rem)
    if tail_sizes is None:
        tail_sizes = []
        r = rem
        while r > 0:
            sz = max(1, r // 2)
            tail_sizes.append(sz)
            r -= sz
        tail_sizes = tuple(tail_sizes)
    for sz in tail_sizes:
        supers.append((m0, sz))
        m0 += sz
    assert sum(ms for _, ms in supers) == nmacro

    nc = bacc.Bacc("TRN2", target_bir_lowering=False, debug=False)

    xt = nc.dram_tensor("xt", [128, 4 * ns_pad], f16, kind="ExternalInput").ap()
    bs = nc.dram_tensor("bs", [ns_pad], f16, kind="ExternalInput").ap()
    wblob = nc.dram_tensor("wblob", [128, 736], f16, kind="ExternalInput").ap()
    ys = nc.dram_tensor("ys", [128, nblk * DTOT], f16,
                        kind="ExternalOutput").ap()

    with tile.TileContext(nc) as tc:
        with tc.tile_pool(name="const", bufs=1) as cpool, \
             tc.tile_pool(name="sbx", bufs=3) as sbx, \
             tc.tile_pool(name="sbo", bufs=6) as sbo, \
             tc.tile_pool(name="sbb", bufs=4) as sbb, \
             tc.tile_pool(name="ps", bufs=7, space="PSUM") as psp, \
             tc.tile_pool(name="psg", bufs=1, space="PSUM") as psg:

            # x loads for a super: issued one super ahead of compute. Super 0
            # is issued first, split across the sync HWDGE queue and the
            # gpsimd SWDGE queue so descriptor generation parallelizes and x
            # data streams from t~=2us; later supers ride gpsimd alone.
            def issue_loads(sup, first=False):
                sm0, msup = sup
                n0 = sm0 * NODES_PER_MACRO
                nn = msup * NODES_PER_MACRO
                xcs = []
                for c in range(4):
                    t = sbx.tile([128, nodes_sup], f16, tag=f"x{c}")
                    eng = nc.sync if (first and c < 2) else nc.gpsimd
                    eng.dma_start(
                        out=t[0:KC[c], 0:nn],
                        in_=xt[0:KC[c], c * ns_pad + n0:c * ns_pad + n0 + nn])
                    xcs.append(t)
                return xcs

            # super 0 loads by hand so the sync queue issues xc0 FIRST (x
            # bytes land ~1.5us earlier), then the tiny consts, then xc1;
            # xc2/xc3 generate concurrently on the gpsimd SWDGE queue
            nn0 = supers[0][1] * NODES_PER_MACRO
            pending = [sbx.tile([128, nodes_sup], f16, tag=f"x{c}",
                                name=f"x{c}") for c in range(4)]
            nc.sync.dma_start(out=pending[0][:, 0:nn0], in_=xt[:, 0:nn0])
            for c in (2, 3):
                nc.gpsimd.dma_start(
                    out=pending[c][0:KC[c], 0:nn0],
                    in_=xt[0:KC[c], c * ns_pad:c * ns_pad + nn0])

            # single fp16 const blob: wc0|wc1|wc2|wc3|w0m|mattr_window
            wcat = cpool.tile([128, 736], f16, tag="wcat")
            nc.sync.dma_start(out=wcat[:], in_=wblob)
            nc.sync.dma_start(out=pending[1][:, 0:nn0],
                              in_=xt[:, ns_pad:ns_pad + nn0])
            # all batch ids up-front (tiny): keeps the one-hot chain off the
            # per-super critical path and off the output DMA queue ordering
            bs_all = cpool.tile([1, ns_pad], f16, tag="bsall")
            nc.sync.dma_start(
                out=bs_all[:],
                in_=bs.rearrange("(one n) -> one n", one=1))

            # ---------------- setup ----------------
            pidx_i = cpool.tile([128, 1], i32, tag="pidxi")
            nc.gpsimd.iota(pidx_i[:], pattern=[[0, 1]], base=0,
                           channel_multiplier=1)
            pidx_f = cpool.tile([128, 1], f32, tag="pidxf")
            nc.vector.tensor_copy(pidx_f[:], pidx_i[:])

            wsb = [wcat[0:KC[c], 128 * c:128 * c + KC[c]] for c in range(4)]

            # G[g, o] = sum_j mattr_win[g, j] * w0m_s[j, o]  (K=2 matmul)
            ps_g = psg.tile([128, 128], f32, tag="g")
            nc.tensor.matmul(ps_g[:], wcat[0:NMOD, 608:736],
                             wcat[0:NMOD, 480:608], start=True, stop=True)
            g16 = cpool.tile([128, 128], f16, tag="g16")
            nc.vector.tensor_copy(g16[:], ps_g[:])

            # ---------------- main loop ----------------
            tail_rr = 0
            for s, (sm0, msup) in enumerate(supers):
                xcs = pending
                if s + 1 < len(supers):
                    pending = issue_loads(supers[s + 1])

                for m in range(msup):
                    gm = sm0 + m                       # global macro index
                    bb = sbb.tile([128, NODES_PER_MACRO], f16, tag="bb")
                    nc.gpsimd.partition_broadcast(
                        bb[:], bs_all[0:1, gm * NODES_PER_MACRO:(gm + 1) * NODES_PER_MACRO])
                    oh = sbb.tile([128, NODES_PER_MACRO], f16, tag="oh")
                    nc.vector.tensor_scalar(oh[:], bb[:], pidx_f[:], None,
                                            op0=mybir.AluOpType.is_equal)

                    # output tiles cover a PAIR of macros: bigger DMA
                    # descriptors (7.7KB/partition) on the store stream
                    if m % 2 == 0:
                        out_sb = sbo.tile([128, 2 * NB * DTOT], f16, tag="out")
                    mo = (m % 2) * NB                  # block offset in pair

                    for q in range(NB):
                        b = NB * m + q           # block within super
                        po = psp.tile([128, DTOT], f32, tag="po")
                        nc.tensor.matmul(
                            po[:, 0:128], xcs[0][:, 128 * b:128 * b + 128],
                            wsb[0], start=True, stop=False,
                            skip_group_check=True)
                        nc.tensor.matmul(
                            po[:, 128:256], xcs[1][:, 128 * b:128 * b + 128],
                            wsb[1], start=False, stop=False,
                            skip_group_check=True)
                        nc.tensor.matmul(
                            po[:, 256:384], xcs[2][:, 128 * b:128 * b + 128],
                            wsb[2], start=False, stop=False,
                            skip_group_check=True)
                        nc.tensor.matmul(
                            po[:, 384:480], xcs[3][0:96, 128 * b:128 * b + 128],
                            wsb[3], start=False, stop=False,
                            skip_group_check=True)
                        nc.tensor.matmul(
                            po[:, 0:128], oh[:, 128 * q:128 * q + 128],
                            g16[:], start=False, stop=True,
                            skip_group_check=True)

                        dst = out_sb[:, DTOT * (mo + q):DTOT * (mo + q + 1)]
                        if q % 2 == 0:
                            nc.vector.tensor_copy(dst, po[:])
                        else:
                            nc.scalar.copy(dst, po[:])

                    if m % 2 == 1 or m == msup - 1:
                        nmac = (m % 2) + 1             # macros in this pair
                        col0 = (gm - nmac + 1) * NB * DTOT
                        # tapered tail supers: spread issue across idle
                        # sequencers so the last stores don't serialize
                        # behind each other's semaphore waits on sync-seq
                        if msup <= 2:
                            eng = (nc.scalar, nc.sync)[tail_rr % 2]
                            tail_rr += 1
                        else:
                            eng = nc.sync
                        eng.dma_start(
                            out=ys[:, col0:col0 + nmac * NB * DTOT],
                            in_=out_sb[:, 0:nmac * NB * DTOT])

    nc.compile()
    return nc


_NC_CACHE = {}


def kernel(x, modal_attr, W0, W0m, W1, W2, batch):
    in_maps, ns, ns_pad = _host_prep(x, modal_attr, W0, W0m, W1, W2, batch)
    if ns_pad not in _NC_CACHE:
        _NC_CACHE[ns_pad] = _build_nc(ns_pad)
    nc = _NC_CACHE[ns_pad]
    res = run_bass_kernel_spmd(nc, in_maps, core_ids=list(range(N_CORES)))
    nblk = ns_pad // 128
    iperm = np.argsort(_perm())
    outs = []
    for i in range(N_CORES):
        ys2 = res.results[i]["ys"]
        op = (ys2.reshape(128, nblk, DTOT).transpose(1, 0, 2)
              .reshape(ns_pad, DTOT)[:ns])
        outs.append(op[:, iperm].astype(np.float32))
    return np.ascontiguousarray(np.concatenate(outs, axis=0))
